# revision 6
# baseline (speedup 1.0000x reference)
"""Trainium2 (8-core SPMD) kernel for the ActorCriticTensorNet MPS head.

reference:
    env0 = einsum('e,eoij->oij', x[0], mps[0])
    for a in 1..63: env = sigmoid(env @ einsum('e,eoij->oij', x[a], mps[a]))
    out = einsum('oii->o', env)

Strategy: the per-agent contractions mat[a] = x[a] . mps[a] are independent;
shard *agents* 8-per-core (tensor streaming dominates: 64 MB of mps per core),
AllGather the 64 tiny (8,32,32) mats (2 MB), then every core redundantly runs
the strictly-sequential 63-step sigmoid chain and the trace epilogue.

The chain keeps env transposed per o-block: with env_T[o][k,i] = env[o][i,k],
    new_env_T[o] = sigmoid( matmul(lhsT=mat[o], rhs=env_T[o]) )
so no per-step transposes are needed; mats are produced in DRAM layout
d = k*256 + j_hi*64 + o*8 + j_lo (k-major) so the chain-side weight views are
simple strided APs and every DMA stays contiguous.
"""

import numpy as np

A, E, O, C = 64, 256, 8, 32
F = O * C * C  # 8192
N_CORES = 8
AL = A // N_CORES  # agents per core

_CACHE = {}


def _build(debug_out=False):
    from concourse import bacc, mybir, tile
    from concourse.masks import make_identity

    F32 = mybir.dt.float32
    nc = bacc.Bacc(
        "TRN2", target_bir_lowering=False, debug=False, num_devices=N_CORES
    )
    x_d = nc.dram_tensor("inputs", [AL, E], F32, kind="ExternalInput")
    mps_d = nc.dram_tensor("mps", [AL, E, F], F32, kind="ExternalInput")
    out_d = nc.dram_tensor("out", [O, 1], F32, kind="ExternalOutput")
    if debug_out:
        matdbg_d = nc.dram_tensor("matdbg", [A, F], F32, kind="ExternalOutput")
        envdbg_d = nc.dram_tensor("envdbg", [C, 256], F32, kind="ExternalOutput")

    with tile.TileContext(nc) as tc:
        with (
            tc.tile_pool(name="dram", bufs=1, space="DRAM") as dram,
            tc.tile_pool(name="mps_pool", bufs=12) as mps_pool,
            tc.tile_pool(name="small", bufs=1) as small,
            tc.tile_pool(name="stage_pool", bufs=4) as stage_pool,
            tc.tile_pool(name="chain_pool", bufs=8) as chain_pool,
            tc.tile_pool(name="env_pool", bufs=2) as env_pool,
            tc.tile_pool(name="ps_mat", bufs=2, space="PSUM") as ps_mat,
            tc.tile_pool(name="ps_chain", bufs=2, space="PSUM") as ps_chain,
            tc.tile_pool(name="ps_misc", bufs=1, space="PSUM") as ps_misc,
        ):
            mat_loc = dram.tile([AL, F], F32)
            mat_all = dram.tile([A, F], F32, addr_space="Shared")

            # x columns: x_sb[e_lo, 2*a + e_hi] = x[a, e_hi*128 + e_lo]
            x_sb = small.tile([128, AL * 2], F32)
            x_view = x_d[:].rearrange("a (eh el u) -> a eh el u", eh=2, el=128, u=1)
            for a in range(AL):
                for eh in range(2):
                    nc.sync.dma_start(
                        x_sb[:, 2 * a + eh : 2 * a + eh + 1], x_view[a, eh]
                    )

            ident = small.tile([C, C], F32)
            make_identity(nc, ident[:])

            # ---- phase 1: mat[a][o,k,j] = sum_e x[a,e] mps[a,e,o,k,j] ----
            # per (a,eh,o) weight tile (128e, 1024f), f = k*32 + j_hi*8 + j_lo.
            # weight column b = o*8 + j_lo reads f = 8*p + j_lo (p = 4k + j_hi)
            # -> a single-free-dim stride-8 AP.  psum[p, b] = mat[o][k][j]
            # with k = p>>2, j = (p&3)*8 + j_lo.
            # DRAM layout d = k*256 + j*8 + o = 64p + (j_lo*8 + o): the
            # psum->stage copy permutes columns (o,jl)->(jl,o) so the store
            # and the chain-side loads are fully contiguous.
            mps_view = mps_d[:].rearrange(
                "a (eh el) (o f) -> a eh el o f", eh=2, el=128, o=O, f=C * C
            )
            mat_loc_v = mat_loc[:].rearrange("a (p b) -> a p b", p=128, b=64)
            for a in range(AL):
                psum_a = ps_mat.tile([128, 64], F32, tag="psa")
                for o in range(O):
                    tvs = []
                    for eh in range(2):
                        t = mps_pool.tile([128, C * C], F32, tag="mps")
                        nc.sync.dma_start(t[:], mps_view[a, eh, :, o, :])
                        tvs.append(
                            t[:].rearrange("e (p jl) -> e p jl", p=128, jl=8)
                        )
                    # each psum column's start/stop matmuls must be
                    # consecutive — interleaved accumulation groups
                    # produce wrong results on HW.
                    for jl in range(8):
                        for eh in range(2):
                            nc.tensor.matmul(
                                psum_a[:, o * 8 + jl : o * 8 + jl + 1],
                                tvs[eh][:, :, jl],
                                x_sb[:, 2 * a + eh : 2 * a + eh + 1],
                                start=(eh == 0),
                                stop=(eh == 1),
                            )
                stage = stage_pool.tile([128, 64], F32, tag="stage")
                nc.vector.tensor_copy(
                    stage[:].rearrange("p (jl o) -> p jl o", jl=8, o=8),
                    psum_a[:].rearrange("p (o jl) -> p jl o", o=8, jl=8),
                )
                nc.sync.dma_start(mat_loc_v[a], stage[:])

            # ---- gather all 64 mats to every core ----
            nc.gpsimd.collective_compute(
                "AllGather",
                mybir.AluOpType.bypass,
                replica_groups=[list(range(N_CORES))],
                ins=[mat_loc.opt()],
                outs=[mat_all.opt()],
            )

            # ---- sequential sigmoid chain over all 64 agents ----
            # mat_all[g] layout d = k*256 + j*8 + o: chain tile is
            # [k, (j,o)]; the per-o weight view [k, j] is a stride-8
            # single-free-dim AP.
            mat_all_v = mat_all[:].rearrange("g (k r) -> g k r", k=C, r=256)

            sb_g = chain_pool.tile([C, 256], F32, tag="chain")
            nc.sync.dma_start(sb_g[:], mat_all_v[0])
            v0 = sb_g[:].rearrange("k (j o) -> k j o", j=C, o=O)
            env_ps = ps_chain.tile([C, 256], F32, tag="cps")
            for o in range(O):
                nc.tensor.transpose(
                    env_ps[:, o * C : (o + 1) * C], v0[:, :, o], ident[:]
                )
            env = env_pool.tile([C, 256], F32, tag="env")
            nc.vector.tensor_copy(env[:], env_ps[:])

            for g in range(1, A):
                sb_g = chain_pool.tile([C, 256], F32, tag="chain")
                nc.sync.dma_start(sb_g[:], mat_all_v[g])
                v = sb_g[:].rearrange("k (j o) -> k j o", j=C, o=O)
                ps_g = ps_chain.tile([C, 256], F32, tag="cps")
                for o in range(O):
                    nc.tensor.matmul(
                        ps_g[:, o * C : (o + 1) * C],
                        v[:, :, o],
                        env[:, o * C : (o + 1) * C],
                        start=True,
                        stop=True,
                    )
                env2 = env_pool.tile([C, 256], F32, tag="env")
                nc.scalar.activation(
                    env2[:], ps_g[:], mybir.ActivationFunctionType.Sigmoid
                )
                env = env2

            if debug_out:
                nc.sync.dma_start(matdbg_d[:], mat_all[:])
                nc.sync.dma_start(envdbg_d[:], env[:])

            # ---- trace epilogue: out[o] = sum_k env_T[o][k, k] ----
            masked = small.tile([C, 256], F32)
            nc.gpsimd.affine_select(
                out=masked[:].rearrange("k (o i) -> k o i", o=O, i=C),
                in_=env[:].rearrange("k (o i) -> k o i", o=O, i=C),
                compare_op=mybir.AluOpType.is_equal,
                fill=0.0,
                base=0,
                pattern=[[0, O], [1, C]],
                channel_multiplier=-1,
            )
            red = small.tile([C, O], F32)
            nc.vector.tensor_reduce(
                red[:].rearrange("k (o u) -> k o u", u=1),
                masked[:].rearrange("k (o i) -> k o i", o=O, i=C),
                axis=mybir.AxisListType.X,
                op=mybir.AluOpType.add,
            )
            ones = small.tile([C, 1], F32)
            nc.gpsimd.memset(ones[:], 1.0)
            pt = ps_misc.tile([O, 1], F32)
            nc.tensor.matmul(pt[:], red[:], ones[:], start=True, stop=True)
            osb = small.tile([O, 1], F32)
            nc.vector.tensor_copy(osb[:], pt[:])
            nc.sync.dma_start(out_d[:], osb[:])

    nc.compile()
    return nc


def get_nc():
    if "nc" not in _CACHE:
        _CACHE["nc"] = _build()
    return _CACHE["nc"]


def make_in_maps(inputs, mps):
    inputs = np.ascontiguousarray(np.asarray(inputs, dtype=np.float32))
    mps = np.ascontiguousarray(
        np.asarray(mps, dtype=np.float32).reshape(A, E, F)
    )
    in_maps = []
    for c in range(N_CORES):
        in_maps.append(
            {
                "inputs": inputs[c * AL : (c + 1) * AL],
                "mps": mps[c * AL : (c + 1) * AL],
            }
        )
    return in_maps


def kernel(inputs, mps):
    from concourse.bass_utils import run_bass_kernel_spmd

    nc = get_nc()
    res = run_bass_kernel_spmd(
        nc, make_in_maps(inputs, mps), core_ids=list(range(N_CORES))
    )
    return res.results[0]["out"].reshape(O).astype(np.float32)


# revision 8
# speedup vs baseline: 2.3204x; 2.3204x over previous
"""Trainium2 (8-core SPMD) kernel for the ActorCriticTensorNet MPS head.

reference:
    env0 = einsum('e,eoij->oij', x[0], mps[0])
    for a in 1..63: env = sigmoid(env @ einsum('e,eoij->oij', x[a], mps[a]))
    out = einsum('oii->o', env)

Strategy: the per-agent contractions mat[a] = x[a] . mps[a] are independent;
shard *agents* 8-per-core (tensor streaming dominates: 64 MB of mps per core),
AllGather the 64 tiny (8,32,32) mats (2 MB), then every core redundantly runs
the strictly-sequential 63-step sigmoid chain and the trace epilogue.

The chain keeps env transposed per o-block: with env_T[o][k,i] = env[o][i,k],
    new_env_T[o] = sigmoid( matmul(lhsT=mat[o], rhs=env_T[o]) )
so no per-step transposes are needed; mats are produced in DRAM layout
d = k*256 + j_hi*64 + o*8 + j_lo (k-major) so the chain-side weight views are
simple strided APs and every DMA stays contiguous.
"""

import numpy as np

A, E, O, C = 64, 256, 8, 32
F = O * C * C  # 8192
N_CORES = 8
AL = A // N_CORES  # agents per core

_CACHE = {}


def _build(debug_out=False):
    from concourse import bacc, mybir, tile
    from concourse.masks import make_identity

    F32 = mybir.dt.float32
    BF16 = mybir.dt.bfloat16
    nc = bacc.Bacc(
        "TRN2", target_bir_lowering=False, debug=False, num_devices=N_CORES
    )
    x_d = nc.dram_tensor("inputs", [AL, E], BF16, kind="ExternalInput")
    mps_d = nc.dram_tensor("mps", [AL, E, F], BF16, kind="ExternalInput")
    out_d = nc.dram_tensor("out", [O, 1], F32, kind="ExternalOutput")
    if debug_out:
        matdbg_d = nc.dram_tensor("matdbg", [A, F], F32, kind="ExternalOutput")
        envdbg_d = nc.dram_tensor("envdbg", [C, 256], F32, kind="ExternalOutput")

    with tile.TileContext(nc) as tc:
        with (
            tc.tile_pool(name="dram", bufs=1, space="DRAM") as dram,
            tc.tile_pool(name="mps_pool", bufs=16) as mps_pool,
            tc.tile_pool(name="small", bufs=1) as small,
            tc.tile_pool(name="stage_pool", bufs=4) as stage_pool,
            tc.tile_pool(name="chain_pool", bufs=8) as chain_pool,
            tc.tile_pool(name="env_pool", bufs=2) as env_pool,
            tc.tile_pool(name="ps_mat", bufs=2, space="PSUM") as ps_mat,
            tc.tile_pool(name="ps_chain", bufs=2, space="PSUM") as ps_chain,
            tc.tile_pool(name="ps_misc", bufs=1, space="PSUM") as ps_misc,
        ):
            mat_loc = dram.tile([AL, F], F32)
            mat_all = dram.tile([A, F], F32, addr_space="Shared")

            # x columns: x_sb[e_lo, 2*a + e_hi] = x[a, e_hi*128 + e_lo]
            x_sb = small.tile([128, AL * 2], BF16)
            x_view = x_d[:].rearrange("a (eh el u) -> a eh el u", eh=2, el=128, u=1)
            for a in range(AL):
                for eh in range(2):
                    nc.sync.dma_start(
                        x_sb[:, 2 * a + eh : 2 * a + eh + 1], x_view[a, eh]
                    )

            ident = small.tile([C, C], F32)
            make_identity(nc, ident[:])

            # ---- phase 1: mat[a][o,k,j] = sum_e x[a,e] mps[a,e,o,k,j] ----
            # per (a,eh,o) weight tile (128e, 1024f), f = k*32 + j_hi*8 + j_lo.
            # weight column b = o*8 + j_lo reads f = 8*p + j_lo (p = 4k + j_hi)
            # -> a single-free-dim stride-8 AP.  psum[p, b] = mat[o][k][j]
            # with k = p>>2, j = (p&3)*8 + j_lo.
            # DRAM layout d = k*256 + j*8 + o = 64p + (j_lo*8 + o): the
            # psum->stage copy permutes columns (o,jl)->(jl,o) so the store
            # and the chain-side loads are fully contiguous.
            mps_view = mps_d[:].rearrange(
                "a (eh el) (o f) -> a eh el o f", eh=2, el=128, o=O, f=C * C
            )
            mat_loc_v = mat_loc[:].rearrange("a (p b) -> a p b", p=128, b=64)
            for a in range(AL):
                psum_a = ps_mat.tile([128, 64], F32, tag="psa")
                for o in range(O):
                    tvs = []
                    for eh in range(2):
                        t = mps_pool.tile([128, C * C], BF16, tag="mps")
                        nc.sync.dma_start(t[:], mps_view[a, eh, :, o, :])
                        tvs.append(
                            t[:].rearrange("e (p jl) -> e p jl", p=128, jl=8)
                        )
                    # each psum column's start/stop matmuls must be
                    # consecutive — interleaved accumulation groups
                    # produce wrong results on HW.
                    for jl in range(8):
                        for eh in range(2):
                            nc.tensor.matmul(
                                psum_a[:, o * 8 + jl : o * 8 + jl + 1],
                                tvs[eh][:, :, jl],
                                x_sb[:, 2 * a + eh : 2 * a + eh + 1],
                                start=(eh == 0),
                                stop=(eh == 1),
                            )
                stage = stage_pool.tile([128, 64], F32, tag="stage")
                nc.vector.tensor_copy(
                    stage[:].rearrange("p (jl o) -> p jl o", jl=8, o=8),
                    psum_a[:].rearrange("p (o jl) -> p jl o", o=8, jl=8),
                )
                nc.sync.dma_start(mat_loc_v[a], stage[:])

            # ---- gather all 64 mats to every core ----
            nc.gpsimd.collective_compute(
                "AllGather",
                mybir.AluOpType.bypass,
                replica_groups=[list(range(N_CORES))],
                ins=[mat_loc.opt()],
                outs=[mat_all.opt()],
            )

            # ---- sequential sigmoid chain over all 64 agents ----
            # mat_all[g] layout d = k*256 + j*8 + o: chain tile is
            # [k, (j,o)]; the per-o weight view [k, j] is a stride-8
            # single-free-dim AP.
            mat_all_v = mat_all[:].rearrange("g (k r) -> g k r", k=C, r=256)

            # two independent o-half pipelines (o 0-3 and 4-7) so the two
            # halves' matmuls/sigmoids interleave on PE/ACT and halve the
            # serial latency of the 63-step chain.
            HO = O // 2  # 4 o's per half

            sb_g = chain_pool.tile([C, 256], F32, tag="chain")
            nc.sync.dma_start(sb_g[:], mat_all_v[0])
            v0 = sb_g[:].rearrange("k (j o) -> k j o", j=C, o=O)
            envs = []
            for h in range(2):
                env_ps = ps_chain.tile([C, HO * C], F32, tag=f"cps{h}")
                for oo in range(HO):
                    nc.tensor.transpose(
                        env_ps[:, oo * C : (oo + 1) * C],
                        v0[:, :, h * HO + oo],
                        ident[:],
                    )
                env_h = env_pool.tile([C, HO * C], F32, tag=f"env{h}")
                nc.vector.tensor_copy(env_h[:], env_ps[:])
                envs.append(env_h)

            for g in range(1, A):
                sb_g = chain_pool.tile([C, 256], F32, tag="chain")
                nc.sync.dma_start(sb_g[:], mat_all_v[g])
                v = sb_g[:].rearrange("k (j o) -> k j o", j=C, o=O)
                for h in range(2):
                    ps_g = ps_chain.tile([C, HO * C], F32, tag=f"cps{h}")
                    for oo in range(HO):
                        nc.tensor.matmul(
                            ps_g[:, oo * C : (oo + 1) * C],
                            v[:, :, h * HO + oo],
                            envs[h][:, oo * C : (oo + 1) * C],
                            start=True,
                            stop=True,
                        )
                    env2 = env_pool.tile([C, HO * C], F32, tag=f"env{h}")
                    nc.scalar.activation(
                        env2[:], ps_g[:], mybir.ActivationFunctionType.Sigmoid
                    )
                    envs[h] = env2

            if debug_out:
                nc.sync.dma_start(matdbg_d[:], mat_all[:])
                nc.sync.dma_start(envdbg_d[:, 0 : HO * C], envs[0][:])
                nc.sync.dma_start(envdbg_d[:, HO * C :], envs[1][:])

            # ---- trace epilogue: out[o] = sum_k env_T[o][k, k] ----
            masked = small.tile([C, 256], F32)
            red = small.tile([C, O], F32)
            for h in range(2):
                mh = masked[:, h * HO * C : (h + 1) * HO * C]
                nc.gpsimd.affine_select(
                    out=mh.rearrange("k (o i) -> k o i", o=HO, i=C),
                    in_=envs[h][:].rearrange("k (o i) -> k o i", o=HO, i=C),
                    compare_op=mybir.AluOpType.is_equal,
                    fill=0.0,
                    base=0,
                    pattern=[[0, HO], [1, C]],
                    channel_multiplier=-1,
                )
                nc.vector.tensor_reduce(
                    red[:, h * HO : (h + 1) * HO].rearrange(
                        "k (o u) -> k o u", u=1
                    ),
                    mh.rearrange("k (o i) -> k o i", o=HO, i=C),
                    axis=mybir.AxisListType.X,
                    op=mybir.AluOpType.add,
                )
            ones = small.tile([C, 1], F32)
            nc.gpsimd.memset(ones[:], 1.0)
            pt = ps_misc.tile([O, 1], F32)
            nc.tensor.matmul(pt[:], red[:], ones[:], start=True, stop=True)
            osb = small.tile([O, 1], F32)
            nc.vector.tensor_copy(osb[:], pt[:])
            nc.sync.dma_start(out_d[:], osb[:])

    nc.compile()
    return nc


def get_nc():
    if "nc" not in _CACHE:
        _CACHE["nc"] = _build()
    return _CACHE["nc"]


def make_in_maps(inputs, mps):
    import ml_dtypes

    inputs = np.ascontiguousarray(
        np.asarray(inputs, dtype=np.float32).astype(ml_dtypes.bfloat16)
    )
    mps = np.ascontiguousarray(
        np.asarray(mps, dtype=np.float32).reshape(A, E, F).astype(ml_dtypes.bfloat16)
    )
    in_maps = []
    for c in range(N_CORES):
        in_maps.append(
            {
                "inputs": inputs[c * AL : (c + 1) * AL],
                "mps": mps[c * AL : (c + 1) * AL],
            }
        )
    return in_maps


def kernel(inputs, mps):
    from concourse.bass_utils import run_bass_kernel_spmd

    nc = get_nc()
    res = run_bass_kernel_spmd(
        nc, make_in_maps(inputs, mps), core_ids=list(range(N_CORES))
    )
    return res.results[0]["out"].reshape(O).astype(np.float32)


# revision 12
# speedup vs baseline: 2.8152x; 1.2132x over previous
"""Trainium2 (8-core SPMD) kernel for the ActorCriticTensorNet MPS head.

reference:
    env0 = einsum('e,eoij->oij', x[0], mps[0])
    for a in 1..63: env = sigmoid(env @ einsum('e,eoij->oij', x[a], mps[a]))
    out = einsum('oii->o', env)

Strategy: the per-agent contractions mat[a] = x[a] . mps[a] are independent;
only the 63-step sigmoid chain is sequential.  Agents are sharded *strided*
(core c holds agents c, c+8, ..., c+56) so that after each core finishes its
g-th local agent, one small AllGather delivers the contiguous global block
[8g, 8g+8) of mats to every core and the sequential chain advances 8 steps —
the chain and the per-group AllGathers overlap the next group's tensor
streaming.  mps/x are converted to bf16 on the host (halves HBM traffic;
PSUM accumulation stays fp32).

The chain keeps env transposed per o-block: with env_T[o][k,i] = env[o][i,k],
    new_env_T[o] = sigmoid( matmul(lhsT=mat[o], rhs=env_T[o]) )
so no per-step transposes are needed.  mats are stored in DRAM layout
d = k*256 + j*8 + o, which makes the phase-1 weight views, the psum->stage
permute, the DRAM stores, the chain loads and the chain weight views all
single-free-dim APs / contiguous DMAs.  The two o-halves of the chain run as
independent pipelines to halve its serial latency.
"""

import numpy as np

A, E, O, C = 64, 256, 8, 32
F = O * C * C  # 8192
N_CORES = 8
AL = A // N_CORES  # agents per core
HO = O // 2

_CACHE = {}


def _build(debug_out=False):
    from concourse import bacc, mybir, tile
    from concourse.masks import make_identity

    F32 = mybir.dt.float32
    BF16 = mybir.dt.bfloat16
    SIG = mybir.ActivationFunctionType.Sigmoid
    nc = bacc.Bacc(
        "TRN2", target_bir_lowering=False, debug=False, num_devices=N_CORES
    )
    x_d = nc.dram_tensor("inputs", [AL, E], BF16, kind="ExternalInput")
    mps_d = nc.dram_tensor("mps", [AL, E, F], BF16, kind="ExternalInput")
    out_d = nc.dram_tensor("out", [O, 1], F32, kind="ExternalOutput")
    if debug_out:
        matdbg_d = nc.dram_tensor("matdbg", [A, F], BF16, kind="ExternalOutput")
        envdbg_d = nc.dram_tensor("envdbg", [C, 256], F32, kind="ExternalOutput")

    with tile.TileContext(nc) as tc:
        with (
            tc.tile_pool(name="dram", bufs=1, space="DRAM") as dram,
            tc.tile_pool(name="mps_pool", bufs=6) as mps_pool,
            tc.tile_pool(name="small", bufs=1) as small,
            tc.tile_pool(name="stage_pool", bufs=4) as stage_pool,
            tc.tile_pool(name="chain_pool", bufs=10) as chain_pool,
            tc.tile_pool(name="env_pool", bufs=2) as env_pool,
            tc.tile_pool(name="ps_mat", bufs=2, space="PSUM") as ps_mat,
            tc.tile_pool(name="ps_chain", bufs=2, space="PSUM") as ps_chain,
            tc.tile_pool(name="ps_misc", bufs=1, space="PSUM") as ps_misc,
        ):
            # AG over one local agent la concatenates the 8 ranks' mats,
            # which with strided sharding is the global block [8la, 8la+8).
            mat_loc = dram.tile([AL, F], BF16)
            mat_alls = [
                dram.tile(
                    [N_CORES, F], BF16, addr_space="Shared", name=f"mat_all{la}"
                )
                for la in range(AL)
            ]
            mat_all_vs = [
                m[:].rearrange("g (k r) -> g k r", k=C, r=256) for m in mat_alls
            ]

            # x columns: x_sb[e_lo, 2*a + e_hi] = x[a, e_hi*128 + e_lo]
            x_sb = small.tile([128, AL * 2], BF16)
            x_view = x_d[:].rearrange("a (eh el u) -> a eh el u", eh=2, el=128, u=1)
            for a in range(AL):
                for eh in range(2):
                    nc.sync.dma_start(
                        x_sb[:, 2 * a + eh : 2 * a + eh + 1], x_view[a, eh]
                    )

            ident = small.tile([C, C], BF16)
            make_identity(nc, ident[:])

            mps_view = mps_d[:].rearrange("a (eh el) f -> a eh el f", eh=2, el=128)
            mat_loc_v = mat_loc[:].rearrange("a (p b) -> a p b", p=128, b=64)

            envs = [None, None]

            def phase1_agent(a):
                # mat[a][o,k,j] = sum_e x[a,e] mps[a,e,o,k,j]
                # weight column b = o*8+jl reads f = o*1024 + 8*p + jl
                # (p = 4k + j_hi) -> stride-8 single-free-dim AP.
                # psum[p, b] = mat[o][k][j], k = p>>2, j = (p&3)*8 + jl.
                psum_a = ps_mat.tile([128, 64], F32, tag="psa")
                tvs = []
                for eh in range(2):
                    t = mps_pool.tile([128, F], BF16, tag="mps")
                    nc.sync.dma_start(t[:], mps_view[a, eh])
                    tvs.append(
                        t[:].rearrange("e (o p jl) -> e o p jl", o=O, p=128, jl=8)
                    )
                # start/stop pairs per psum column must be consecutive
                # (interleaved accumulation groups compute wrong results
                # on HW).
                for o in range(O):
                    for jl in range(8):
                        for eh in range(2):
                            nc.tensor.matmul(
                                psum_a[:, o * 8 + jl : o * 8 + jl + 1],
                                tvs[eh][:, o, :, jl],
                                x_sb[:, 2 * a + eh : 2 * a + eh + 1],
                                start=(eh == 0),
                                stop=(eh == 1),
                            )
                # DRAM layout d = k*256 + j*8 + o = 64p + (jl*8 + o):
                # permute columns (o,jl)->(jl,o) on the way out of PSUM.
                stage = stage_pool.tile([128, 64], BF16, tag="stage")
                nc.vector.tensor_copy(
                    stage[:].rearrange("p (jl o) -> p jl o", jl=8, o=8),
                    psum_a[:].rearrange("p (o jl) -> p jl o", o=8, jl=8),
                )
                nc.sync.dma_start(mat_loc_v[a], stage[:])

            def chain_step(g, init=False):
                sb_g = chain_pool.tile([C, 256], BF16, tag="chain")
                nc.sync.dma_start(sb_g[:], mat_all_vs[g // 8][g % 8])
                v = sb_g[:].rearrange("k (j o) -> k j o", j=C, o=O)
                for h in range(2):
                    # is_transpose matmuls require psum dtype == input dtype
                    ps_g = ps_chain.tile(
                        [C, HO * C], BF16 if init else F32, tag=f"cps{h}"
                    )
                    for oo in range(HO):
                        o = h * HO + oo
                        if init:
                            nc.tensor.transpose(
                                ps_g[:, oo * C : (oo + 1) * C],
                                v[:, :, o],
                                ident[:],
                            )
                        else:
                            nc.tensor.matmul(
                                ps_g[:, oo * C : (oo + 1) * C],
                                v[:, :, o],
                                envs[h][:, oo * C : (oo + 1) * C],
                                start=True,
                                stop=True,
                            )
                    env2 = env_pool.tile([C, HO * C], BF16, tag=f"env{h}")
                    if init:
                        nc.vector.tensor_copy(env2[:], ps_g[:])
                    else:
                        nc.scalar.activation(env2[:], ps_g[:], SIG)
                    envs[h] = env2

            for la in range(AL):
                phase1_agent(la)
                nc.gpsimd.collective_compute(
                    "AllGather",
                    mybir.AluOpType.bypass,
                    replica_groups=[list(range(N_CORES))],
                    ins=[mat_loc[la : la + 1, :].opt()],
                    outs=[mat_alls[la].opt()],
                )
                for g in range(la * 8, la * 8 + 8):
                    chain_step(g, init=(g == 0))

            if debug_out:
                dbg = small.tile([C, 256], F32)
                nc.vector.tensor_copy(dbg[:, 0 : HO * C], envs[0][:])
                nc.vector.tensor_copy(dbg[:, HO * C :], envs[1][:])
                nc.sync.dma_start(envdbg_d[:], dbg[:])
                for la in range(AL):
                    nc.sync.dma_start(
                        matdbg_d[la * 8 : (la + 1) * 8, :], mat_alls[la][:]
                    )

            # ---- trace epilogue: out[o] = sum_k env_T[o][k, k] ----
            masked = small.tile([C, 256], BF16)
            red = small.tile([C, O], F32)
            for h in range(2):
                mh = masked[:, h * HO * C : (h + 1) * HO * C]
                nc.gpsimd.affine_select(
                    out=mh.rearrange("k (o i) -> k o i", o=HO, i=C),
                    in_=envs[h][:].rearrange("k (o i) -> k o i", o=HO, i=C),
                    compare_op=mybir.AluOpType.is_equal,
                    fill=0.0,
                    base=0,
                    pattern=[[0, HO], [1, C]],
                    channel_multiplier=-1,
                )
                nc.vector.tensor_reduce(
                    red[:, h * HO : (h + 1) * HO].rearrange(
                        "k (o u) -> k o u", u=1
                    ),
                    mh.rearrange("k (o i) -> k o i", o=HO, i=C),
                    axis=mybir.AxisListType.X,
                    op=mybir.AluOpType.add,
                )
            ones = small.tile([C, 1], F32)
            nc.gpsimd.memset(ones[:], 1.0)
            pt = ps_misc.tile([O, 1], F32)
            nc.tensor.matmul(pt[:], red[:], ones[:], start=True, stop=True)
            osb = small.tile([O, 1], F32)
            nc.vector.tensor_copy(osb[:], pt[:])
            nc.sync.dma_start(out_d[:], osb[:])

    nc.compile()
    return nc


def get_nc():
    if "nc" not in _CACHE:
        _CACHE["nc"] = _build()
    return _CACHE["nc"]


def make_in_maps(inputs, mps):
    import ml_dtypes

    inputs = np.asarray(inputs, dtype=np.float32).astype(ml_dtypes.bfloat16)
    mps = (
        np.asarray(mps, dtype=np.float32)
        .reshape(A, E, F)
        .astype(ml_dtypes.bfloat16)
    )
    in_maps = []
    for c in range(N_CORES):
        # strided sharding: core c holds global agents c, c+8, ..., c+56
        in_maps.append(
            {
                "inputs": np.ascontiguousarray(inputs[c::N_CORES]),
                "mps": np.ascontiguousarray(mps[c::N_CORES]),
            }
        )
    return in_maps


def kernel(inputs, mps):
    from concourse.bass_utils import run_bass_kernel_spmd

    nc = get_nc()
    res = run_bass_kernel_spmd(
        nc, make_in_maps(inputs, mps), core_ids=list(range(N_CORES))
    )
    return res.results[0]["out"].reshape(O).astype(np.float32)
